# revision 1
# baseline (speedup 1.0000x reference)
# Trainium2 Bass kernel for the MindForge LoRA head problem — v2.
#
# Key structure (vs the v1 baseline):
#   * All matmul operands bf16 (host-side conversion): halves HBM traffic,
#     guarantees full PE rate (1 cycle/row).
#   * Single pass over base_w: full x^T resident in SBUF (64KB/partition).
#   * Host-side DRAM tiling: every DMA is a contiguous 8-16KB run per
#     partition (128 descriptors per transfer), maximizing DMA-engine
#     efficiency.
#   * DMAs spread round-robin over the three available queues
#     (qSPDynamicHW via nc.sync, qActDynamicHW via nc.scalar,
#     qPoolDynamic via nc.gpsimd) — a single queue sustains only ~15-21
#     GB/s on HW.
#   * Coefficient pipeline fused per b-tile so uT33 slices become
#     available early; x loaded in 8 chunks so phase B can start before
#     the full x^T load completes.
#
# Computation (see reference):
#   h0      = context @ ctx_w.T + ctx_b          (B, H)
#   h       = gelu(LN(h0) * ln_g + ln_b)         (B, H)
#   coeffs  = h @ coeff_w.T + coeff_b            (B, 8)
#   y       = x @ A_flat.T                       (B, 32)
#   z_br    = sum_n coeffs_bn * y_b(n,r)         (B, 4)
#   u       = coeffs_bn * z_br                   (B, 32)
#   out     = x @ base_w.T + base_b + u @ Bmat   (B, C)
#
# Distribution: column-parallel over num_classes; each of 8 cores owns a
# CS=6400-wide padded shard. base_b is folded into the LoRA matmul as a
# 33rd row of [u | 1] @ [Bmat ; base_b].

import numpy as np
from contextlib import ExitStack

import ml_dtypes
import concourse.bass as bass
import concourse.tile as tile
from concourse import bacc, mybir
from concourse.bass_utils import run_bass_kernel_spmd
from concourse.masks import make_identity

F32 = mybir.dt.float32
BF16 = mybir.dt.bfloat16
AF = mybir.ActivationFunctionType
AX = mybir.AxisListType

D = 2048          # d_model
B = 2048          # batch
C_FULL = 50257    # num_classes
NB = 8            # n_basis
RK = 4            # rank
H = 128           # hidden
N_CORES = 8
CS = 6288         # per-core padded class shard (8*6288 = 50304 >= 50257)
LN_EPS = 1e-5

KT = D // 128            # 16 k-tiles
BT = B // 128            # 16 b-tiles
CW = 256                 # batch chunk width for x/ctx loads
NCH = B // CW            # 8 chunks
C_TILES = [512] * 12 + [144]   # sums to 6288
assert sum(C_TILES) == CS
POFF = np.cumsum([0] + [KT * w for w in C_TILES]).tolist()   # bw panel offsets
OOFF = np.cumsum([0] + [BT * w for w in C_TILES]).tolist()   # out block offsets

# index-table column offsets (int16 gather indices, [16, IDX_COLS])
IDX_BW = 0               # 12 panels x 8
IDX_X = 12 * 8           # 8 x-chunks x 8
IDX_T = IDX_X + 8 * 8    # tail panel x 8
IDX_COLS = IDX_T + 8


def _make_idxg():
    idx = np.zeros((16, IDX_COLS), np.int16)
    for ci in range(12):
        for i in range(128):
            idx[i % 16, IDX_BW + ci * 8 + i // 16] = ci * 128 + i
    for bc in range(NCH):
        for i in range(128):
            idx[i % 16, IDX_X + bc * 8 + i // 16] = bc * 128 + i
    for i in range(128):
        idx[i % 16, IDX_T + i // 16] = i
    # gather reads the table from a [128, n] AP (content wrapped in the
    # first 16 partitions) — replicate for safety
    return np.ascontiguousarray(np.tile(idx, (8, 1)))


def _gq(g):
    # SWDGE queue for the next Pool DMA: must follow the tile framework's
    # DMASW sem-lane rotation (lane i is locked to one queue) — pattern
    # [1,2,3,0] keeps lane->queue assignment consistent (8 lanes, period 4).
    i = g["poolctr"][0]
    g["poolctr"][0] += 1
    return [1, 2, 3, 0][i % 4]


def _emit_rep(nc, P, g, u):
    """Emit one full repetition of the kernel body."""
    d_x, d_ctx, d_bw, d_out = g["d_x"], g["d_ctx"], g["d_bw"], g["d_out"]
    ident, cwT, aT, coefw, ctxb, lng, lnb, coefb, bm = (
        g["ident"], g["cwT"], g["aT"], g["coefw"], g["ctxb"], g["lng"],
        g["lnb"], g["coefb"], g["bm"])
    idxg = g["idxg"]
    CHB = KT * CW            # columns per x/ctx chunk

    # Per-chunk x tiles and per-t uT33 tiles: fine-grained cross-rep
    # dependencies let rep N+1's loads chase rep N's tail instead of
    # waiting for its last matmul.
    xts = [P["xpool"].tile([128, CHB], BF16, tag=f"xt{bc}",
                           name=f"xt_{u}_{bc}") for bc in range(NCH)]
    uts = [P["upool"].tile([NB * RK + 1, 128], BF16, tag=f"uT{t}",
                           name=f"uT_{u}_{t}") for t in range(BT)]

    # ===== phase A: coefficient pipeline, fused per b-tile =====
    for t in range(BT):
        bc, j = divmod(t, CW // 128)
        if j == 0:
            # x and ctx chunks gathered on rotating SWDGE queues
            nc.gpsimd.dma_gather(
                xts[bc][:].unsqueeze(1),
                d_x[:, :],
                idxg[:, IDX_X + bc * 8:IDX_X + (bc + 1) * 8],
                num_idxs=128, num_idxs_reg=128, elem_size=CHB,
                queue_num=_gq(g))
            ct = P["cstr"].tile([128, CHB], BF16, tag="ct", name=f"ct_{u}_{bc}")
            nc.gpsimd.dma_gather(
                ct[:].unsqueeze(1),
                d_ctx[:, :],
                idxg[:, IDX_X + bc * 8:IDX_X + (bc + 1) * 8],
                num_idxs=128, num_idxs_reg=128, elem_size=CHB,
                queue_num=_gq(g))

        # A1: h0_t [128b, H] = sum_k ctx^T[k, t-block]^T @ ctx_wT[k] (+ ctx_b)
        acc = P["psA"].tile([128, H], F32, tag="accA", name=f"h0ps_{u}_{t}")
        for k in range(KT):
            nc.tensor.matmul(acc[:],
                             ct[:, k * CW + j * 128:k * CW + (j + 1) * 128],
                             cwT[:, k * H:(k + 1) * H],
                             start=(k == 0), stop=(k == KT - 1))
        h0 = P["sbA"].tile([128, H], F32, tag="h0", name=f"h0_{u}_{t}")
        nc.vector.tensor_add(h0[:], acc[:], ctxb[:])

        # A2: y_t [128b, 32] = sum_k x^T[k, t-block]^T @ A_flat^T[k]
        acy = P["psA"].tile([128, 32], F32, tag="accA", name=f"yps_{u}_{t}")
        for k in range(KT):
            nc.tensor.matmul(acy[:],
                             xts[bc][:, k * CW + j * 128:k * CW + (j + 1) * 128],
                             aT[:, k * 32:(k + 1) * 32],
                             start=(k == 0), stop=(k == KT - 1))
        yt = P["sbA"].tile([128, 32], F32, tag="yt", name=f"yt_{u}_{t}")
        nc.vector.tensor_copy(yt[:], acy[:])

        # A3: LayerNorm + gelu (batch on partitions)
        mu = P["small"].tile([128, 1], F32, tag="mu", name=f"mu_{u}_{t}")
        s2 = P["small"].tile([128, 1], F32, tag="s2", name=f"s2_{u}_{t}")
        sq = P["sbA"].tile([128, H], F32, tag="sq", name=f"sq_{u}_{t}")
        nc.vector.reduce_sum(mu[:], h0[:], axis=AX.X)
        nc.scalar.activation(sq[:], h0[:], AF.Square, accum_out=s2[:])
        nc.vector.tensor_scalar_mul(mu[:], mu[:], 1.0 / H)
        nc.vector.tensor_scalar_mul(s2[:], s2[:], 1.0 / H)
        mu2 = P["small"].tile([128, 1], F32, tag="mu2", name=f"mu2_{u}_{t}")
        nc.vector.tensor_mul(mu2[:], mu[:], mu[:])
        var = P["small"].tile([128, 1], F32, tag="var", name=f"var_{u}_{t}")
        nc.vector.tensor_sub(var[:], s2[:], mu2[:])
        nc.vector.tensor_scalar_add(var[:], var[:], LN_EPS)
        std = P["small"].tile([128, 1], F32, tag="std", name=f"std_{u}_{t}")
        nc.scalar.sqrt(std[:], var[:])
        rstd = P["small"].tile([128, 1], F32, tag="rstd", name=f"rstd_{u}_{t}")
        nc.vector.reciprocal(rstd[:], std[:])
        nmr = P["small"].tile([128, 1], F32, tag="nmr", name=f"nmr_{u}_{t}")
        nc.vector.tensor_mul(nmr[:], mu[:], rstd[:])
        nc.vector.tensor_scalar_mul(nmr[:], nmr[:], -1.0)
        hn = P["sbA"].tile([128, H], F32, tag="hn", name=f"hn_{u}_{t}")
        nc.scalar.activation(hn[:], h0[:], AF.Identity, bias=nmr[:], scale=rstd[:])
        nc.vector.tensor_mul(hn[:], hn[:], lng[:])
        nc.vector.tensor_add(hn[:], hn[:], lnb[:])
        nc.scalar.activation(h0[:], hn[:], AF.Gelu)

        # A4: transpose h -> hT (bf16), coeffs = h @ coeff_w^T + coeff_b
        trh = P["psA"].tile([128, 128], F32, tag="accA", name=f"htr_{u}_{t}")
        nc.tensor.transpose(trh[:], h0[:], ident[:])
        hT = P["sbA"].tile([128, 128], BF16, tag="hT", name=f"hT_{u}_{t}")
        nc.vector.tensor_copy(hT[:], trh[:])
        cfp = P["psA"].tile([128, NB], F32, tag="accA", name=f"cfps_{u}_{t}")
        nc.tensor.matmul(cfp[:], hT[:], coefw[:], start=True, stop=True)
        cf = P["sbA"].tile([128, NB], F32, tag="cf", name=f"cf_{u}_{t}")
        nc.vector.tensor_add(cf[:], cfp[:], coefb[:])

        # A5: z = sum_n coeffs*y ; u = coeffs (x) z ; uT33[:, t] = [u | 1]^T
        prod = P["sbA"].tile([128, 32], F32, tag="prod", name=f"prod_{u}_{t}")
        # prod stored r-major: prod[p, r*8+n] = y[p, n*4+r] * coeffs[p, n]
        nc.vector.tensor_mul(
            prod[:].rearrange("p (r n) -> p r n", n=NB),
            yt[:].rearrange("p (n r) -> p r n", r=RK),
            cf[:].unsqueeze(1).broadcast_to((128, RK, NB)))
        z = P["small"].tile([128, RK], F32, tag="z", name=f"z_{u}_{t}")
        nc.vector.reduce_sum(z[:], prod[:].rearrange("p (r n) -> p r n", n=NB),
                             axis=AX.X)
        ut = P["sbA"].tile([128, NB * RK + 1], F32, tag="ut", name=f"ut_{u}_{t}")
        nc.vector.tensor_mul(
            ut[:, :NB * RK].rearrange("p (n r) -> p n r", r=RK),
            cf[:].unsqueeze(2).broadcast_to((128, NB, RK)),
            z[:].unsqueeze(1).broadcast_to((128, NB, RK)))
        nc.gpsimd.memset(ut[:, NB * RK:NB * RK + 1], 1.0)
        tru = P["psA"].tile([NB * RK + 1, 128], F32, tag="accA", name=f"utr_{u}_{t}")
        nc.tensor.transpose(tru[:], ut[:], ident[:])
        nc.vector.tensor_copy(uts[t][:], tru[:])

    # ===== phase B: main column-parallel matmul =====
    QS = [nc.sync, nc.scalar]
    c0 = 0
    for ci, W in enumerate(C_TILES):
        panel = P["bwp"].tile([128, KT * W], BF16, tag="bw", name=f"bw_{u}_{ci}")
        if W == 512:
            nc.gpsimd.dma_gather(
                panel[:].unsqueeze(1),
                d_bw[:, :],
                idxg[:, IDX_BW + ci * 8:IDX_BW + (ci + 1) * 8],
                num_idxs=128, num_idxs_reg=128, elem_size=KT * W,
                queue_num=_gq(g))
        else:
            nc.gpsimd.dma_gather(
                panel[:].unsqueeze(1),
                g["d_bwt"][:, :],
                idxg[:, IDX_T:IDX_T + 8],
                num_idxs=128, num_idxs_reg=128, elem_size=KT * W,
                queue_num=_gq(g))
        otC = P["outp"].tile([128, BT * W], BF16, tag="ot", name=f"ot_{u}_{ci}")
        for b in range(BT):
            bc, j = divmod(b, CW // 128)
            po = P["psM"].tile([128, W], F32, tag="out", name=f"po_{u}_{ci}_{b}")
            for k in range(KT):
                nc.tensor.matmul(
                    po[:],
                    xts[bc][:, k * CW + j * 128:k * CW + (j + 1) * 128],
                    panel[:, k * W:(k + 1) * W],
                    start=(k == 0), stop=False)
            nc.tensor.matmul(po[:], uts[b][:],
                             bm[:, c0:c0 + W], start=False, stop=True)
            nc.vector.tensor_copy(otC[:, b * W:(b + 1) * W], po[:])
        QS[ci % 2].dma_start(d_out[:, OOFF[ci]:OOFF[ci + 1]], otC[:])
        c0 += W


def _build_program(reps=1):
    nc = bacc.Bacc("TRN2", target_bir_lowering=False, debug=False,
                   num_devices=N_CORES, num_swdge_queues=4)

    g = {"poolctr": [0]}
    # DRAM I/O — host-tiled layouts (see prepare_in_maps)
    g["d_x"] = nc.dram_tensor("x_g", [NCH * 128, KT * CW], BF16,
                              kind="ExternalInput").ap()
    g["d_ctx"] = nc.dram_tensor("ctx_g", [NCH * 128, KT * CW], BF16,
                                kind="ExternalInput").ap()
    d_idx = nc.dram_tensor("idxg", [128, IDX_COLS], mybir.dt.int16,
                           kind="ExternalInput").ap()
    d_cwT = nc.dram_tensor("cw_t", [128, KT * H], BF16, kind="ExternalInput").ap()
    d_aT = nc.dram_tensor("a_t", [128, KT * NB * RK], BF16,
                          kind="ExternalInput").ap()
    d_ctx_b = nc.dram_tensor("ctx_b_bc", [128, H], F32, kind="ExternalInput").ap()
    d_ln_g = nc.dram_tensor("ln_g_bc", [128, H], F32, kind="ExternalInput").ap()
    d_ln_b = nc.dram_tensor("ln_b_bc", [128, H], F32, kind="ExternalInput").ap()
    d_cw = nc.dram_tensor("coeff_wT", [H, NB], BF16, kind="ExternalInput").ap()
    d_cb = nc.dram_tensor("coeff_b_bc", [128, NB], F32, kind="ExternalInput").ap()
    g["d_bw"] = nc.dram_tensor("bw_g", [12 * 128, KT * 512], BF16,
                               kind="ExternalInput").ap()
    g["d_bwt"] = nc.dram_tensor("bwt_g", [128, KT * C_TILES[12]], BF16,
                                kind="ExternalInput").ap()
    d_Bm = nc.dram_tensor("Bm33", [NB * RK + 1, CS], BF16,
                          kind="ExternalInput").ap()
    g["d_out"] = nc.dram_tensor("out", [128, BT * CS], BF16,
                                kind="ExternalOutput").ap()

    with tile.TileContext(nc) as tc, ExitStack() as ctx:
        P = {}
        P["const"] = ctx.enter_context(tc.tile_pool(name="const", bufs=1))
        P["sbA"] = ctx.enter_context(tc.tile_pool(name="sbA", bufs=2))
        P["upool"] = ctx.enter_context(tc.tile_pool(name="upool", bufs=2))
        P["small"] = ctx.enter_context(tc.tile_pool(name="small", bufs=4))
        P["cstr"] = ctx.enter_context(tc.tile_pool(name="cstr", bufs=2))
        P["xpool"] = ctx.enter_context(tc.tile_pool(name="xpool", bufs=1))
        P["bwp"] = ctx.enter_context(tc.tile_pool(name="bwp", bufs=3))
        P["outp"] = ctx.enter_context(tc.tile_pool(name="outp", bufs=2))
        P["psA"] = ctx.enter_context(tc.tile_pool(name="psA", bufs=2, space="PSUM"))
        P["psM"] = ctx.enter_context(tc.tile_pool(name="psM", bufs=6, space="PSUM"))

        # ---- constants / replicated small tensors (one-time loads) ----
        ident = P["const"].tile([128, 128], F32, name="ident")
        make_identity(nc, ident[:])
        g["ident"] = ident
        cwT = P["const"].tile([128, KT * H], BF16, name="cwT")
        nc.sync.dma_start(cwT[:], d_cwT[:])
        g["cwT"] = cwT
        aT = P["const"].tile([128, KT * NB * RK], BF16, name="aT")
        nc.sync.dma_start(aT[:], d_aT[:])
        g["aT"] = aT
        coefw = P["const"].tile([H, NB], BF16, name="coefw")
        nc.sync.dma_start(coefw[:], d_cw[:, :])
        g["coefw"] = coefw
        ctxb = P["const"].tile([128, H], F32, name="ctxb")
        nc.sync.dma_start(ctxb[:], d_ctx_b[:, :])
        g["ctxb"] = ctxb
        lng = P["const"].tile([128, H], F32, name="lng")
        nc.sync.dma_start(lng[:], d_ln_g[:, :])
        g["lng"] = lng
        lnb = P["const"].tile([128, H], F32, name="lnb")
        nc.sync.dma_start(lnb[:], d_ln_b[:, :])
        g["lnb"] = lnb
        coefb = P["const"].tile([128, NB], F32, name="coefb")
        nc.sync.dma_start(coefb[:], d_cb[:, :])
        g["coefb"] = coefb
        bm = P["const"].tile([NB * RK + 1, CS], BF16, name="bm")
        nc.scalar.dma_start(bm[:], d_Bm[:, :])
        g["bm"] = bm
        idxg = P["const"].tile([128, IDX_COLS], mybir.dt.int16, name="idxg")
        nc.sync.dma_start(idxg[:], d_idx[:, :])
        g["idxg"] = idxg

        for rep in range(reps):
            _emit_rep(nc, P, g, f"{rep}")

    nc.compile()

    # The tile scheduler assigns each Pool DMA a DMASW sem lane (proc) in
    # *scheduled* order; the runtime locks each lane's semaphore to a single
    # SWDGE queue. Rewrite queue_num from the assigned lane so the mapping
    # is consistent: lane i -> queue i % 4.
    from concourse.tile_sem_assignment import PROC_NAME_TO_IDX
    lane_by_proc = {PROC_NAME_TO_IDX[f"DMASW{i}"]: i for i in range(8)}
    fn = nc.m.functions[0]
    for blk in (fn.blocks if isinstance(fn.blocks, list) else [fn.blocks]):
        for inst in blk.instructions:
            if isinstance(inst, mybir.InstDMAGatherAnt):
                lane = lane_by_proc.get(inst.bass_scheduled_proc)
                if lane is not None:
                    inst.queue_num = lane % 4
    return nc


_NC = None


def _get_program():
    global _NC
    if _NC is None:
        _NC = _build_program()
    return _NC


def _bf(a):
    return np.asarray(np.asarray(a, np.float32), dtype=ml_dtypes.bfloat16)


def _tile_km(mat_T, ncols_inner):
    """[D, N] (k*128+p, n) -> [128, KT*N] with layout (p, k-major blocks).

    ncols_inner: inner column width per k-block (N)."""
    Dn, N = mat_T.shape
    assert Dn == D and N == ncols_inner
    return np.ascontiguousarray(
        mat_T.reshape(KT, 128, N).transpose(1, 0, 2).reshape(128, KT * N))


def prepare_in_maps(x, context, base_w, base_b, ctx_w, ctx_b, ln_g, ln_b,
                    coeff_w, coeff_b, basis_A, basis_B):
    x = np.asarray(x, np.float32)
    context = np.asarray(context, np.float32)
    base_w = np.asarray(base_w, np.float32)
    base_b = np.asarray(base_b, np.float32)
    ctx_b = np.asarray(ctx_b, np.float32)
    ln_g = np.asarray(ln_g, np.float32)
    ln_b = np.asarray(ln_b, np.float32)
    basis_A = np.asarray(basis_A, np.float32)
    basis_B = np.asarray(basis_B, np.float32)

    # x / ctx: [128, (bc, k, bi)] so one 8KB-contiguous chunk per b-chunk
    def tile_xc(m):
        # m: [B, D] -> mT [D, B] -> [k, p, bc, bi] -> [p, bc, k, bi]
        mT = _bf(m.T)
        return np.ascontiguousarray(
            mT.reshape(KT, 128, NCH, CW).transpose(1, 2, 0, 3).reshape(128, -1))

    # x / ctx as gather tables: row bc*128+p holds chunk bc's partition-p run
    def gtable(m):
        return np.ascontiguousarray(
            tile_xc(m).reshape(128, NCH, KT * CW).transpose(1, 0, 2)
            .reshape(NCH * 128, KT * CW))

    x_g = gtable(x)
    ctx_g = gtable(context)
    cw_t = _tile_km(_bf(np.asarray(ctx_w, np.float32).T), H)
    a_t = _tile_km(_bf(basis_A.transpose(2, 0, 1).reshape(D, NB * RK)), NB * RK)
    ctx_b_bc = np.ascontiguousarray(np.broadcast_to(ctx_b[None, :], (128, H)))
    ln_g_bc = np.ascontiguousarray(np.broadcast_to(ln_g[None, :], (128, H)))
    ln_b_bc = np.ascontiguousarray(np.broadcast_to(ln_b[None, :], (128, H)))
    coeff_wT = np.ascontiguousarray(_bf(np.asarray(coeff_w, np.float32).T))
    coeff_b_bc = np.ascontiguousarray(
        np.broadcast_to(np.asarray(coeff_b, np.float32)[None, :], (128, NB)))

    C_PAD = N_CORES * CS
    bwT = np.zeros((D, C_PAD), ml_dtypes.bfloat16)
    bwT[:, :C_FULL] = _bf(base_w.T)
    Bm33 = np.zeros((NB * RK + 1, C_PAD), ml_dtypes.bfloat16)
    Bm33[:NB * RK, :C_FULL] = _bf(
        basis_B.transpose(0, 2, 1).reshape(NB * RK, C_FULL))
    Bm33[NB * RK, :C_FULL] = _bf(base_b)

    rep = {
        "x_g": x_g, "ctx_g": ctx_g, "cw_t": cw_t, "a_t": a_t,
        "ctx_b_bc": ctx_b_bc, "ln_g_bc": ln_g_bc, "ln_b_bc": ln_b_bc,
        "coeff_wT": coeff_wT, "coeff_b_bc": coeff_b_bc,
        "idxg": _make_idxg(),
    }
    in_maps = []
    for c in range(N_CORES):
        sl = slice(c * CS, (c + 1) * CS)
        m = dict(rep)
        shard = bwT[:, sl]
        # bw_g: gather table — row ci*128+p holds panel ci's partition-p run
        blocks = []
        cc = 0
        for W in C_TILES:
            seg = np.ascontiguousarray(shard[:, cc:cc + W])
            blocks.append(
                seg.reshape(KT, 128, W).transpose(1, 0, 2).reshape(128, KT * W))
            cc += W
        m["bw_g"] = np.ascontiguousarray(
            np.stack(blocks[:12], axis=0).reshape(12 * 128, KT * 512))
        m["bwt_g"] = np.ascontiguousarray(blocks[12])
        m["Bm33"] = np.ascontiguousarray(Bm33[:, sl])
        in_maps.append(m)
    return in_maps


def _unshuffle_out(raw):
    """raw [128, BT*CS] (per-ci blocks of [p, (t, w)]) -> [B, CS] f32."""
    out = np.empty((B, CS), np.float32)
    cc = 0
    for ci, W in enumerate(C_TILES):
        blk = np.asarray(raw[:, OOFF[ci]:OOFF[ci + 1]], dtype=np.float32)
        out[:, cc:cc + W] = (
            blk.reshape(128, BT, W).transpose(1, 0, 2).reshape(B, W))
        cc += W
    return out


def run(in_maps, **spmd_kwargs):
    nc = _get_program()
    res = run_bass_kernel_spmd(nc, in_maps, core_ids=list(range(N_CORES)),
                               **spmd_kwargs)
    out = np.concatenate(
        [_unshuffle_out(res.results[c]["out"]) for c in range(N_CORES)], axis=1)
    return np.ascontiguousarray(out[:, :C_FULL]), res


def kernel(**inputs):
    in_maps = prepare_in_maps(**inputs)
    out, _ = run(in_maps)
    return out



# revision 2
# speedup vs baseline: 1.0847x; 1.0847x over previous
# Trainium2 Bass kernel for the MindForge LoRA head problem — v7.
#
# Structure (vs the v2 baseline at ~866us single-dispatch):
#   * All-bf16 matmuls (an fp8-DoubleRow hybrid was tried and reverted:
#     the presence of DR matmuls in the NEFF clocks the PE domain at
#     5/6 speed — a net loss at any hybrid fraction).
#   * All DMA via plain dma_start on the two HWDGE rings (sync/scalar)
#     with explicit per-ring FIFO ordering. The v2 SWDGE gather path
#     started its first transfer ~28us in (index-table dep + ~1.6us
#     Q7 descriptor emission per gather); HWDGE measures ~250GB/s.
#   * The x @ A_flat projection (y) is fused into the tail class panel
#     as 32 extra columns — removes 256 small N=32 matmuls.
#   * Tail c-tile runs FIRST, interleaved chunk-wise with the
#     coefficient pipeline and the first main c-tile, so the PE has
#     dense work from ~2us on; ~20 junk warmup matmuls on cwT flip HAM
#     to K=8/8 before real data lands (v2 spent ~79us cold/starved).
#   * LoRA delta: u is 32 rows; per-b-tile delta matmuls run 4-way
#     row-tiled (tile_position=(32i,0)) — ~4x fewer PE-serial slots.
#     base_b is added by the DVE during the PSUM->SBUF copy.
#   * u^T transposes on DVE 32x32 stream transposes (off the PE).
#
# Distribution: column-parallel over num_classes; each of 8 cores owns
# a CS=6288-wide padded shard (8*6288 = 50304 >= 50257).

import numpy as np
from contextlib import ExitStack

import ml_dtypes
import concourse.bass as bass
import concourse.tile as tile
from concourse import bacc, mybir
from concourse.bass_utils import run_bass_kernel_spmd
from concourse.masks import make_identity

F32 = mybir.dt.float32
BF16 = mybir.dt.bfloat16
AF = mybir.ActivationFunctionType
AX = mybir.AxisListType

D = 2048          # d_model
B = 2048          # batch
C_FULL = 50257    # num_classes
NB = 8            # n_basis
RK = 4            # rank
H = 128           # hidden
N_CORES = 8
CS = 6288         # per-core padded class shard
LN_EPS = 1e-5

KT = D // 128            # 16 k-tiles
BT = B // 128            # 16 b-tiles
CW = 256                 # batch chunk width for x/ctx loads
NCH = B // CW            # 8 chunks
CHB = KT * CW            # 4096 chunk columns
W_TAIL = 144             # tail class tile
A_COLS = NB * RK         # 32 fused A_flat columns
TAILW = W_TAIL + A_COLS  # 176
C_TILES = [512] * 12 + [W_TAIL]
assert sum(C_TILES) == CS
C0_TAIL = 12 * 512
OOFF = np.cumsum([0] + [BT * w for w in C_TILES]).tolist()   # out block offsets
N_JUNK = 12              # HAM warmup matmuls (first rep only, fp32 N=512)


def _tail_chain(nc, P, g, u, t, pot, otT):
    """Tail c-tile chain for b-tile t: [bw_tail | A_flat] columns."""
    bc, j = divmod(t, 2)
    for k in range(KT):
        nc.tensor.matmul(pot[:],
                         g["xts"][bc][:, k * CW + j * 128:k * CW + (j + 1) * 128],
                         g["bwt"][:, k * TAILW:(k + 1) * TAILW],
                         start=(k == 0), stop=(k == KT - 1))
    # y_t for the coefficient pipeline; classes part -> otT (bias and
    # LoRA delta are added later, off the tail critical path)
    yt = P["sbA"].tile([128, A_COLS], F32, tag="yt", name=f"yt_{u}_{t}",
                       bufs=5)
    nc.vector.tensor_copy(yt[:], pot[:, W_TAIL:TAILW])
    nc.vector.tensor_copy(otT[:, t * W_TAIL:(t + 1) * W_TAIL], pot[:, :W_TAIL])
    return yt


def _phase_a1(nc, P, g, u, t):
    """Coefficient pipeline part 1 for b-tile t: h0 -> LN -> gelu.
    Returns the h (post-gelu) tile; the PE-side part 2 (_phase_a2) is
    emitted later so the DVE/ACT chain here never stalls the PE."""
    cwT, ctxb, lng, lnb = g["cwT"], g["ctxb"], g["lng"], g["lnb"]
    bc, j = divmod(t, 2)
    ct = g["cts"][bc]

    # h0_t [128b, H] = ctx-block^T @ cwT (+ ctx_b)
    acc = P["psA"].tile([128, H], F32, tag="accA", name=f"h0ps_{u}_{t}")
    for k in range(KT):
        nc.tensor.matmul(acc[:],
                         ct[:, k * CW + j * 128:k * CW + (j + 1) * 128],
                         cwT[:, k * H:(k + 1) * H],
                         start=(k == 0), stop=(k == KT - 1))
    h0 = P["sbA"].tile([128, H], F32, tag="h0", name=f"h0_{u}_{t}",
                   bufs=5)
    nc.vector.tensor_add(h0[:], acc[:], ctxb[:])

    # LayerNorm + gelu (batch on partitions)
    mu = P["small"].tile([128, 1], F32, tag="mu", name=f"mu_{u}_{t}")
    s2 = P["small"].tile([128, 1], F32, tag="s2", name=f"s2_{u}_{t}")
    sq = P["sbA"].tile([128, H], F32, tag="sq", name=f"sq_{u}_{t}")
    nc.vector.reduce_sum(mu[:], h0[:], axis=AX.X)
    nc.scalar.activation(sq[:], h0[:], AF.Square, accum_out=s2[:])
    nc.vector.tensor_scalar_mul(mu[:], mu[:], 1.0 / H)
    nc.vector.tensor_scalar_mul(s2[:], s2[:], 1.0 / H)
    mu2 = P["small"].tile([128, 1], F32, tag="mu2", name=f"mu2_{u}_{t}")
    nc.vector.tensor_mul(mu2[:], mu[:], mu[:])
    var = P["small"].tile([128, 1], F32, tag="var", name=f"var_{u}_{t}")
    nc.vector.tensor_sub(var[:], s2[:], mu2[:])
    nc.vector.tensor_scalar_add(var[:], var[:], LN_EPS)
    std = P["small"].tile([128, 1], F32, tag="std", name=f"std_{u}_{t}")
    nc.scalar.sqrt(std[:], var[:])
    rstd = P["small"].tile([128, 1], F32, tag="rstd", name=f"rstd_{u}_{t}")
    nc.vector.reciprocal(rstd[:], std[:])
    nmr = P["small"].tile([128, 1], F32, tag="nmr", name=f"nmr_{u}_{t}")
    nc.vector.tensor_mul(nmr[:], mu[:], rstd[:])
    nc.vector.tensor_scalar_mul(nmr[:], nmr[:], -1.0)
    hn = P["sbA"].tile([128, H], F32, tag="hn", name=f"hn_{u}_{t}")
    nc.scalar.activation(hn[:], h0[:], AF.Identity, bias=nmr[:], scale=rstd[:])
    nc.vector.tensor_mul(hn[:], hn[:], lng[:])
    nc.vector.tensor_add(hn[:], hn[:], lnb[:])
    nc.scalar.activation(h0[:], hn[:], AF.Gelu)
    return h0


def _phase_a2(nc, P, g, u, t, h0, yt, upk):
    """Coefficient pipeline part 2: coeffs -> u -> upk strip."""
    ident, coefw, coefb = g["ident"], g["coefw"], g["coefb"]
    # coeffs = h @ coeff_w^T + coeff_b  (via PE transpose of h)
    trh = P["psA"].tile([128, 128], F32, tag="accA", name=f"htr_{u}_{t}")
    nc.tensor.transpose(trh[:], h0[:], ident[:])
    hT = P["sbA"].tile([128, 128], BF16, tag="hT", name=f"hT_{u}_{t}")
    nc.vector.tensor_copy(hT[:], trh[:])
    cfp = P["psA"].tile([128, NB], F32, tag="accA", name=f"cfps_{u}_{t}")
    nc.tensor.matmul(cfp[:], hT[:], coefw[:], start=True, stop=True)
    cf = P["sbA"].tile([128, NB], F32, tag="cf", name=f"cf_{u}_{t}")
    nc.vector.tensor_add(cf[:], cfp[:], coefb[:])

    # z = sum_n coeffs*y ; u = coeffs (x) z  (32 rows, bf16)
    prod = P["sbA"].tile([128, 32], F32, tag="prod", name=f"prod_{u}_{t}")
    nc.vector.tensor_mul(
        prod[:].rearrange("p (r n) -> p r n", n=NB),
        yt[:].rearrange("p (n r) -> p r n", r=RK),
        cf[:].unsqueeze(1).broadcast_to((128, RK, NB)))
    z = P["small"].tile([128, RK], F32, tag="z", name=f"z_{u}_{t}")
    nc.vector.reduce_sum(z[:], prod[:].rearrange("p (r n) -> p r n", n=NB),
                         axis=AX.X)
    ut = P["sbA"].tile([128, A_COLS], BF16, tag="ut", name=f"ut_{u}_{t}")
    nc.vector.tensor_mul(
        ut[:].rearrange("p (n r) -> p n r", r=RK),
        cf[:].unsqueeze(2).broadcast_to((128, NB, RK)),
        z[:].unsqueeze(1).broadcast_to((128, NB, RK)))
    # transpose into upk strip 32*(t%4) via DVE 32x32 blocks
    i = t % 4
    for jb in range(4):
        nc.vector.transpose(
            upk[32 * i:32 * (i + 1), 32 * jb:32 * (jb + 1)],
            ut[32 * jb:32 * (jb + 1), :])


def _main_chains(nc, P, g, u, ci, bg, panel, W, irange):
    """k-chains for b-tiles 4*bg+i, i in irange, of main c-tile ci."""
    pos = []
    for i in irange:
        b = 4 * bg + i
        bc, j = divmod(b, 2)
        po = P["psM"].tile([128, W], F32, tag="out", name=f"po_{u}_{ci}_{b}")
        for k in range(KT):
            nc.tensor.matmul(
                po[:],
                g["xts"][bc][:, k * CW + j * 128:k * CW + (j + 1) * 128],
                panel[:, k * W:(k + 1) * W],
                start=(k == 0), stop=False)
        pos.append(po)
    return pos


def _main_lora_adds(nc, P, g, u, ci, bg, pos, otC, W, c0, dq=None):
    """4-way row-tiled LoRA + bias adds closing b-group bg of c-tile ci.
    If dq is given, also DMA the finished otC b-group slice out."""
    upk = g["upks"][bg]
    for i in range(4):
        nc.tensor.matmul(pos[i][:],
                         upk[32 * i:32 * (i + 1), :],
                         g["bm"][32 * i:32 * (i + 1), c0:c0 + W],
                         start=False, stop=True, tile_position=(32 * i, 0))
    for i in range(4):
        b = 4 * bg + i
        nc.vector.tensor_add(otC[:, b * W:(b + 1) * W], pos[i][:],
                             g["bias"][:, c0:c0 + W])
    if dq is not None:
        lo, hi = 4 * bg * W, 4 * (bg + 1) * W
        dq.dma_start(g["d_out"][:, OOFF[ci] + lo:OOFF[ci] + hi],
                     otC[:, lo:hi])


def _main_bgroup(nc, P, g, u, ci, bg, panel, otC, W, c0, dq=None):
    pos = _main_chains(nc, P, g, u, ci, bg, panel, W, range(4))
    _main_lora_adds(nc, P, g, u, ci, bg, pos, otC, W, c0, dq=dq)


def _emit_rep(nc, P, g, u):
    d_x, d_ctx, d_bw, d_out = g["d_x"], g["d_ctx"], g["d_bw"], g["d_out"]
    QS = [nc.sync, nc.scalar]

    # ---- HAM warmup (first rep only): fp32 junk matmuls on a memset
    # tile — no HBM dependency, ~850ns each warm, so ~12 of them carry
    # the PE from the end of the runtime preamble (~7us) to the arrival
    # of the first x chunk (~21us)
    if u == "0":
        junkt = P["const"].tile([128, 512], F32, name="junkt")
        nc.gpsimd.memset(junkt[:], 1.0)
        for w in range(N_JUNK):
            jp = P["psA"].tile([128, 512], F32, tag="accA", name=f"junk_{w}")
            nc.tensor.matmul(jp[:], junkt[:, :128], junkt[:],
                             start=True, stop=True)

    xts = [P["xpool"].tile([128, CHB], BF16, tag=f"xt{bc}",
                           name=f"xt_{u}_{bc}") for bc in range(NCH)]
    cts = [P["cstr"].tile([128, CHB], BF16, tag="ct", name=f"ct_{u}_{bc}")
           for bc in range(NCH)]
    g["xts"], g["cts"] = xts, cts
    upks = [P["upool"].tile([128, 128], BF16, tag=f"up{bg}",
                            name=f"up_{u}_{bg}") for bg in range(4)]
    g["upks"] = upks

    # explicit by-need ring schedule for the startup-critical loads
    XRING = {0: 0, 1: 1, 2: 1, 3: 0, 4: 0, 5: 1, 6: 0, 7: 1}
    CRING = {0: 0, 1: 1, 2: 1, 3: 0, 4: 1, 5: 0, 6: 1, 7: 0}

    def load_x(bc):
        QS[XRING[bc]].dma_start(xts[bc][:], d_x[bc * 128:(bc + 1) * 128, :])

    def load_ctx(bc):
        QS[CRING[bc]].dma_start(cts[bc][:], d_ctx[bc * 128:(bc + 1) * 128, :])

    def load_panel(ci, split=False):
        panel = P["bwp"].tile([128, KT * 512], BF16, tag="bw",
                              name=f"bw_{u}_{ci}")
        if split:
            half = KT * 256
            nc.sync.dma_start(panel[:, :half],
                              d_bw[ci * 128:(ci + 1) * 128, :half])
            nc.scalar.dma_start(panel[:, half:],
                                d_bw[ci * 128:(ci + 1) * 128, half:])
        else:
            QS[ci % 2].dma_start(panel[:], d_bw[ci * 128:(ci + 1) * 128, :])
        return panel

    # startup loads, by need: tail panel + x0/x1 (~19us), ctx0/ctx1
    # (~25us), panel0 split over both rings (~30us), then bm/bias
    bwt = P["bwp"].tile([128, KT * TAILW], BF16, tag="bwt", name=f"bwt_{u}")
    nc.sync.dma_start(bwt[:], g["d_bwt"][:, :])
    g["bwt"] = bwt
    load_x(0)
    load_x(1)
    load_ctx(0)
    load_ctx(1)
    panels = {0: load_panel(0, split=True)}

    # ---- tail c-tile + coefficient pipeline + main ci=0, chunk-paced.
    # Each b-group's LoRA close is deferred past the next group's tail
    # chains so the DVE coefficient chain never gates the PE. ----
    otT = P["outp"].tile([128, BT * W_TAIL], BF16, tag="otT", name=f"otT_{u}")
    otC0 = P["outp"].tile([128, BT * 512], BF16, tag="ot", name=f"ot_{u}_0")
    prev_pos = None
    for bg in range(4):
        if bg == 1 and u == "0":
            # bm/bias: small [32, CS] HBM loads + on-chip doubling
            nc.sync.dma_start(g["bm"][0:32, :], g["d_bm"][:, :])
            nc.sync.dma_start(g["bm"][32:64, :], g["bm"][0:32, :])
            nc.sync.dma_start(g["bm"][64:128, :], g["bm"][0:64, :])
            nc.scalar.dma_start(g["bias"][0:32, :], g["d_bias"][:, :])
            nc.scalar.dma_start(g["bias"][32:64, :], g["bias"][0:32, :])
            nc.scalar.dma_start(g["bias"][64:128, :], g["bias"][0:64, :])
        for bc in (2 * bg + 2, 2 * bg + 3):
            if bc < NCH:
                load_ctx(bc)
        for bc in (2 * bg + 2, 2 * bg + 3):
            if bc < NCH:
                load_x(bc)
        yts = {}
        h0s = {}
        # bg>=1: A1s first so the DVE/ACT coefficient chain starts ~5us
        # earlier (bg0's ctx lands after x, so tails lead there)
        if bg > 0:
            for i in range(4):
                t = 4 * bg + i
                h0s[t] = _phase_a1(nc, P, g, u, t)
        for i in range(4):
            t = 4 * bg + i
            pot = P["psA"].tile([128, TAILW], F32, tag="accA",
                                name=f"pot_{u}_{t}")
            yts[t] = _tail_chain(nc, P, g, u, t, pot, otT)
        if prev_pos is not None:
            _main_lora_adds(nc, P, g, u, 0, bg - 1, prev_pos, otC0, 512, 0,
                            dq=QS[0])
        if bg == 0:
            for i in range(4):
                t = 4 * bg + i
                h0s[t] = _phase_a1(nc, P, g, u, t)
        pos = _main_chains(nc, P, g, u, 0, bg, panels[0], 512, range(2))
        for i in range(4):
            t = 4 * bg + i
            _phase_a2(nc, P, g, u, t, h0s[t], yts[t], upks[bg])
        pos += _main_chains(nc, P, g, u, 0, bg, panels[0], 512, range(2, 4))
        prev_pos = pos
    _main_lora_adds(nc, P, g, u, 0, 3, prev_pos, otC0, 512, 0, dq=QS[0])
    panels[1] = load_panel(1)

    # ---- tail LoRA delta + bias + otT out ----
    for t in range(BT):
        i = t % 4
        dtl = P["psA"].tile([128, W_TAIL], F32, tag="accA", name=f"dtl_{u}_{t}")
        nc.tensor.matmul(dtl[:],
                         g["upks"][t // 4][32 * i:32 * (i + 1), :],
                         g["bm"][32 * i:32 * (i + 1), C0_TAIL:C0_TAIL + W_TAIL],
                         start=True, stop=True, tile_position=(32 * i, 0))
        nc.vector.tensor_add(otT[:, t * W_TAIL:(t + 1) * W_TAIL],
                             otT[:, t * W_TAIL:(t + 1) * W_TAIL], dtl[:])
        nc.vector.tensor_add(otT[:, t * W_TAIL:(t + 1) * W_TAIL],
                             otT[:, t * W_TAIL:(t + 1) * W_TAIL],
                             g["bias"][:, C0_TAIL:C0_TAIL + W_TAIL])
    QS[1].dma_start(d_out[:, OOFF[12]:OOFF[13]], otT[:])

    # ---- main c-tiles 1..11 ----
    for ci in range(1, 12):
        if ci + 1 < 12:
            panels[ci + 1] = load_panel(ci + 1)
        otC = P["outp"].tile([128, BT * 512], BF16, tag="ot", name=f"ot_{u}_{ci}")
        for bg in range(4):
            _main_bgroup(nc, P, g, u, ci, bg, panels[ci], otC, 512, ci * 512,
                         dq=QS[ci % 2])


def _build_program(reps=1):
    nc = bacc.Bacc("TRN2", target_bir_lowering=False, debug=False,
                   num_devices=N_CORES)

    g = {}
    g["d_x"] = nc.dram_tensor("x_g", [NCH * 128, CHB], BF16,
                              kind="ExternalInput").ap()
    g["d_ctx"] = nc.dram_tensor("ctx_g", [NCH * 128, CHB], BF16,
                                kind="ExternalInput").ap()
    d_cwT = nc.dram_tensor("cw_t", [128, KT * H], BF16, kind="ExternalInput").ap()
    d_ctx_b = nc.dram_tensor("ctx_b_bc", [128, H], F32, kind="ExternalInput").ap()
    d_ln_g = nc.dram_tensor("ln_g_bc", [128, H], F32, kind="ExternalInput").ap()
    d_ln_b = nc.dram_tensor("ln_b_bc", [128, H], F32, kind="ExternalInput").ap()
    d_cw = nc.dram_tensor("coeff_wT", [H, NB], BF16, kind="ExternalInput").ap()
    d_cb = nc.dram_tensor("coeff_b_bc", [128, NB], F32, kind="ExternalInput").ap()
    g["d_bw"] = nc.dram_tensor("bw_g", [12 * 128, KT * 512], BF16,
                               kind="ExternalInput").ap()
    g["d_bwt"] = nc.dram_tensor("bwt_g", [128, KT * TAILW], BF16,
                                kind="ExternalInput").ap()
    g["d_bm"] = nc.dram_tensor("bm32", [32, CS], BF16,
                               kind="ExternalInput").ap()
    g["d_bias"] = nc.dram_tensor("bias_g", [32, CS], BF16,
                                 kind="ExternalInput").ap()
    g["d_out"] = nc.dram_tensor("out", [128, BT * CS], BF16,
                                kind="ExternalOutput").ap()

    with tile.TileContext(nc) as tc, ExitStack() as ctx:
        P = {}
        P["const"] = ctx.enter_context(tc.tile_pool(name="const", bufs=1))
        P["sbA"] = ctx.enter_context(tc.tile_pool(name="sbA", bufs=2))
        P["upool"] = ctx.enter_context(tc.tile_pool(name="upool", bufs=2))
        P["small"] = ctx.enter_context(tc.tile_pool(name="small", bufs=4))
        P["cstr"] = ctx.enter_context(tc.tile_pool(name="cstr", bufs=2))
        P["xpool"] = ctx.enter_context(tc.tile_pool(name="xpool", bufs=1))
        P["bwp"] = ctx.enter_context(tc.tile_pool(name="bwp", bufs=2))
        P["outp"] = ctx.enter_context(tc.tile_pool(name="outp", bufs=2))
        P["psA"] = ctx.enter_context(tc.tile_pool(name="psA", bufs=2, space="PSUM"))
        P["psM"] = ctx.enter_context(tc.tile_pool(name="psM", bufs=6, space="PSUM"))

        # ---- constants (cwT first: the warmup matmuls depend on it) ----
        cwT = P["const"].tile([128, KT * H], BF16, name="cwT")
        nc.scalar.dma_start(cwT[:], d_cwT[:])
        g["cwT"] = cwT
        ident = P["const"].tile([128, 128], F32, name="ident")
        make_identity(nc, ident[:])
        g["ident"] = ident
        coefw = P["const"].tile([H, NB], BF16, name="coefw")
        nc.scalar.dma_start(coefw[:], d_cw[:, :])
        g["coefw"] = coefw
        ctxb = P["const"].tile([128, H], F32, name="ctxb")
        nc.scalar.dma_start(ctxb[:], d_ctx_b[:, :])
        g["ctxb"] = ctxb
        lng = P["const"].tile([128, H], F32, name="lng")
        nc.scalar.dma_start(lng[:], d_ln_g[:, :])
        g["lng"] = lng
        lnb = P["const"].tile([128, H], F32, name="lnb")
        nc.scalar.dma_start(lnb[:], d_ln_b[:, :])
        g["lnb"] = lnb
        coefb = P["const"].tile([128, NB], F32, name="coefb")
        nc.scalar.dma_start(coefb[:], d_cb[:, :])
        g["coefb"] = coefb
        # bm/bias tiles allocated here, loaded inside rep 0 (ring order)
        g["bm"] = P["const"].tile([128, CS], BF16, name="bm")
        g["bias"] = P["const"].tile([128, CS], BF16, name="bias")

        for rep in range(reps):
            _emit_rep(nc, P, g, f"{rep}")

    nc.compile()
    return nc


_NC = None


def _get_program():
    global _NC
    if _NC is None:
        _NC = _build_program()
    return _NC


def _bf(a):
    return np.asarray(np.asarray(a, np.float32), dtype=ml_dtypes.bfloat16)


def prepare_in_maps(x, context, base_w, base_b, ctx_w, ctx_b, ln_g, ln_b,
                    coeff_w, coeff_b, basis_A, basis_B):
    x = np.asarray(x, np.float32)
    context = np.asarray(context, np.float32)
    base_w = np.asarray(base_w, np.float32)
    base_b = np.asarray(base_b, np.float32)
    ctx_b = np.asarray(ctx_b, np.float32)
    ln_g = np.asarray(ln_g, np.float32)
    ln_b = np.asarray(ln_b, np.float32)
    basis_A = np.asarray(basis_A, np.float32)
    basis_B = np.asarray(basis_B, np.float32)

    def chunk_table(m):
        mT = _bf(m.T)
        return np.ascontiguousarray(
            mT.reshape(KT, 128, NCH, CW).transpose(2, 1, 0, 3)
            .reshape(NCH * 128, CHB))

    x_g = chunk_table(x)
    ctx_g = chunk_table(context)

    cw_t = _bf(ctx_w.T).reshape(KT, 128, H).transpose(1, 0, 2).reshape(128, -1)
    cw_t = np.ascontiguousarray(cw_t)
    ctx_b_bc = np.ascontiguousarray(np.broadcast_to(ctx_b[None, :], (128, H)))
    ln_g_bc = np.ascontiguousarray(np.broadcast_to(ln_g[None, :], (128, H)))
    ln_b_bc = np.ascontiguousarray(np.broadcast_to(ln_b[None, :], (128, H)))
    coeff_wT = np.ascontiguousarray(_bf(np.asarray(coeff_w, np.float32).T))
    coeff_b_bc = np.ascontiguousarray(
        np.broadcast_to(np.asarray(coeff_b, np.float32)[None, :], (128, NB)))

    A_flat = basis_A.transpose(2, 0, 1).reshape(D, NB * RK)   # [D, 32]
    C_PAD = N_CORES * CS
    bwT = np.zeros((D, C_PAD), np.float32)
    bwT[:, :C_FULL] = base_w.T
    Bmat = np.zeros((A_COLS, C_PAD), np.float32)
    Bmat[:, :C_FULL] = basis_B.transpose(0, 2, 1).reshape(A_COLS, C_FULL)
    bias_full = np.zeros((C_PAD,), np.float32)
    bias_full[:C_FULL] = base_b

    rep = {
        "x_g": x_g, "ctx_g": ctx_g, "cw_t": cw_t,
        "ctx_b_bc": ctx_b_bc, "ln_g_bc": ln_g_bc, "ln_b_bc": ln_b_bc,
        "coeff_wT": coeff_wT, "coeff_b_bc": coeff_b_bc,
    }
    in_maps = []
    for c in range(N_CORES):
        sl = slice(c * CS, (c + 1) * CS)
        m = dict(rep)
        shard = bwT[:, sl]                        # [D, CS] f32
        blocks = []
        for ci in range(12):
            seg = _bf(shard[:, ci * 512:(ci + 1) * 512])
            blocks.append(
                seg.reshape(KT, 128, 512).transpose(1, 0, 2)
                .reshape(128, KT * 512))
        m["bw_g"] = np.ascontiguousarray(
            np.stack(blocks, axis=0).reshape(12 * 128, KT * 512))
        tailm = _bf(np.concatenate(
            [shard[:, C0_TAIL:CS], A_flat], axis=1))   # [D, 176]
        m["bwt_g"] = np.ascontiguousarray(
            tailm.reshape(KT, 128, TAILW).transpose(1, 0, 2)
            .reshape(128, KT * TAILW))
        m["bm32"] = np.ascontiguousarray(_bf(Bmat[:, sl]))
        m["bias_g"] = np.ascontiguousarray(
            np.broadcast_to(_bf(bias_full[sl])[None, :], (32, CS)))
        in_maps.append(m)
    return in_maps


def _unshuffle_out(raw):
    """raw [128, BT*CS] (per-ci blocks of [p, (t, w)]) -> [B, CS] f32."""
    out = np.empty((B, CS), np.float32)
    cc = 0
    for ci, W in enumerate(C_TILES):
        blk = np.asarray(raw[:, OOFF[ci]:OOFF[ci + 1]], dtype=np.float32)
        out[:, cc:cc + W] = (
            blk.reshape(128, BT, W).transpose(1, 0, 2).reshape(B, W))
        cc += W
    return out


def run(in_maps, **spmd_kwargs):
    nc = _get_program()
    res = run_bass_kernel_spmd(nc, in_maps, core_ids=list(range(N_CORES)),
                               **spmd_kwargs)
    out = np.concatenate(
        [_unshuffle_out(res.results[c]["out"]) for c in range(N_CORES)], axis=1)
    return np.ascontiguousarray(out[:, :C_FULL]), res


def kernel(**inputs):
    in_maps = prepare_in_maps(**inputs)
    out, _ = run(in_maps)
    return out
